# revision 5
# baseline (speedup 1.0000x reference)
"""DeltaSynapse message-passing kernel for Trainium2 (8 NeuronCores).

Computes I = einsum('eo,dbe,deo,dbe->bo', signs*W, Xd, delaymap, Wshort+1)
with the post dimension (o) sharded across 8 cores.

Math note: reference signs = where(W>0, 2*signs_pre-1, 0) and W >= 0, so
signs*W == (2*signs_pre-1)*W exactly. Fold the sign vector and the fp8
descale into the small tensor A'[d,b,e] = Xd*(Wshort+1)*s[e]/SW, and fold
W into the delay-routing map on the host:
    M8[d,e,o] = e3m4(SW * W[e,o]) * delaymap[d,e,o]     (fp8, 1 B/elem)
    I[b,o]    = sum_{d,e} A'[d,b,e] * M8[d,e,o]

Per-core plan (o-shard of 256 columns):
  - M8 shard (8 x 2048 x 256 fp8e3 = 4MB) is built host-side (delaymap is
    binary so the mask costs no precision; W is quantized to e3m4 with a
    x512 scale so values sit in the e3m4 normal range; max rel err 2^-5).
    Streamed as 8 x 0.5MB per-delay DMAs (4KB per partition, contiguous),
    alternating the two HWDGE queues (sync/scalar).
  - A' (128 x 2048 bf16 = 0.5MB) is built host-side and DMA'd once.
  - The PE runs 128 accumulating matmuls (K=128 e's, M=16 batch, N=256
    posts) into one PSUM tile: lhsT = A'[:, c, d, :] (bf16), rhs =
    M8[:, c, :] (fp8e3). Mixed bf16 x fp8 is legal (fp22 internal) and
    streams at 1 col/cycle, so PE ~= 128*256 cycles ~= 13.7us warm.
  - No DVE/ACT work in the main loop; only the PSUM->SBUF output copy.
"""

import numpy as np

import concourse.bass as bass  # noqa: F401
import concourse.mybir as mybir
from concourse import bacc
from concourse.bass_utils import run_bass_kernel_spmd
from concourse.tile import TileContext

D, B, E, O = 8, 16, 2048, 2048
NCORES = 8
P = 128
O_SH = O // NCORES  # 256 post columns per core
EC = E // P  # 16 e-chunks
SW = 512.0  # fp8 weight scale (folded back via A' /= SW)

_NC_CACHE = {}


def _build(loop_iters=None):
    f32 = mybir.dt.float32
    bf16 = mybir.dt.bfloat16
    fp8 = mybir.dt.float8e3

    nc = bacc.Bacc("TRN2", target_bir_lowering=False, debug=False)
    x_m8 = nc.dram_tensor("m8", [D, P, EC * O_SH], fp8, kind="ExternalInput")
    x_a = nc.dram_tensor("a", [P, EC * D * B], bf16, kind="ExternalInput")
    y = nc.dram_tensor("y", [B, O_SH], f32, kind="ExternalOutput")

    with TileContext(nc) as tc:
        with (
            tc.tile_pool(name="ap_", bufs=2) as ap_,
            tc.tile_pool(name="m8p", bufs=8) as m8p,
            tc.tile_pool(name="psp", bufs=2, space="PSUM") as psp,
            tc.tile_pool(name="outp", bufs=2) as outp,
        ):

            def body(_i=None):
                # lhsT source: a[p, c, d, b]
                a_t = ap_.tile([P, EC, D, B], bf16, tag="a")
                nc.scalar.dma_start(out=a_t[:], in_=x_a.ap())

                ps = psp.tile([B, O_SH], f32, tag="ps")
                # d-pairs: 1MB DMAs (78% of peak vs ~73% at 0.5MB)
                for dp in range(D // 2):
                    m_t = m8p.tile([P, 2, EC, O_SH], fp8, tag="m8")
                    eng = nc.scalar if dp % 2 else nc.sync
                    src = x_m8.ap()[2 * dp : 2 * dp + 2]
                    eng.dma_start(
                        out=m_t[:], in_=src.rearrange("d p x -> p d x")
                    )
                    for dd in range(2):
                        d = 2 * dp + dd
                        for c in range(EC):
                            nc.tensor.matmul(
                                ps[:],
                                a_t[:, c, d, :],
                                m_t[:, dd, c, :],
                                start=(d == 0 and c == 0),
                                stop=(d == D - 1 and c == EC - 1),
                            )

                o_t = outp.tile([B, O_SH], f32, tag="o")
                nc.vector.tensor_copy(out=o_t[:], in_=ps[:])
                nc.scalar.dma_start(out=y.ap(), in_=o_t[:])

            if loop_iters is None:
                body()
            else:
                # Unroll 2 bodies per hardware-loop iteration: inside For_i
                # the body's SBUF addresses are fixed, so pools only rotate
                # across unrolled copies — without this, iteration i+1's
                # input DMAs serialize behind iteration i's full PE chain.
                assert loop_iters % 2 == 0
                with tc.For_i(
                    0, loop_iters // 2, 1, hint_engines=(mybir.EngineType.PE,)
                ) as i:
                    body(i)
                    body(i)

    nc.compile()
    return nc


def _get_nc(loop_iters=None):
    key = loop_iters
    if key not in _NC_CACHE:
        _NC_CACHE[key] = _build(loop_iters)
    return _NC_CACHE[key]


def _make_in_maps(W, Xd, delaymap, Wshort, signs_pre):
    import ml_dtypes

    bf16 = ml_dtypes.bfloat16
    e3m4 = ml_dtypes.float8_e3m4
    W = np.asarray(W, dtype=np.float32)
    Xd = np.asarray(Xd, dtype=np.float32)
    delaymap = np.asarray(delaymap)
    Wshort = np.asarray(Wshort, dtype=np.float32)
    signs_pre = np.asarray(signs_pre)

    s = (2 * signs_pre - 1).astype(np.float32)  # (E,)
    # A'[d,b,e] = Xd*(Wshort+1)*s/SW; layout a[p, c, d, b]
    a = (Xd * (Wshort + 1.0)) * (s / SW)[None, None, :]  # (D,B,E)
    a_re = np.ascontiguousarray(
        a.reshape(D, B, EC, P).transpose(3, 2, 0, 1).reshape(P, EC * D * B)
    ).astype(bf16)

    # W quantized once to e3m4 (scaled), then masked per-delay (exact).
    w8 = (W * SW).astype(e3m4).astype(np.float32)  # (E,O)
    dm_b = delaymap != 0  # (D,E,O) binary

    in_maps = []
    for i in range(NCORES):
        o0 = i * O_SH
        m8 = np.where(dm_b[:, :, o0 : o0 + O_SH], w8[None, :, o0 : o0 + O_SH], 0.0)
        # layout m8[d, p, (c, o)]
        m8_re = np.ascontiguousarray(
            m8.reshape(D, EC, P, O_SH)
            .transpose(0, 2, 1, 3)
            .reshape(D, P, EC * O_SH)
            .astype(e3m4)
        )
        in_maps.append({"m8": m8_re, "a": a_re})
    return in_maps


def run(W, Xd, delaymap, Wshort, signs_pre, loop_iters=None):
    """Run on the 8 NeuronCores; returns (I, BassKernelResults)."""
    nc = _get_nc(loop_iters)
    in_maps = _make_in_maps(W, Xd, delaymap, Wshort, signs_pre)
    res = run_bass_kernel_spmd(nc, in_maps, core_ids=list(range(NCORES)))
    I = np.concatenate(
        [res.results[i]["y"] for i in range(NCORES)], axis=1
    ).astype(np.float32)
    return I, res


def kernel(W, Xd, delaymap, Wshort, signs_pre):
    I, _ = run(W, Xd, delaymap, Wshort, signs_pre)
    return I


# revision 6
# speedup vs baseline: 1.2612x; 1.2612x over previous
"""DeltaSynapse message-passing kernel for Trainium2 (8 NeuronCores).

Computes I = einsum('eo,dbe,deo,dbe->bo', signs*W, Xd, delaymap, Wshort+1)
with the post dimension (o) sharded across 8 cores.

Math note: reference signs = where(W>0, 2*signs_pre-1, 0) and W >= 0, so
signs*W == (2*signs_pre-1)*W exactly. Fold the sign vector and the fp8
descale into the small tensor A'[d,b,e] = Xd*(Wshort+1)*s[e]/SW, and fold
W into the delay-routing map on the host:
    M8[d,e,o] = e3m4(SW * W[e,o]) * delaymap[d,e,o]     (fp8, 1 B/elem)
    I[b,o]    = sum_{d,e} A'[d,b,e] * M8[d,e,o]

Per-core plan (o-shard of 256 columns):
  - M8 shard (8 x 2048 x 256 fp8e3 = 4MB) is built host-side (delaymap is
    binary so the mask costs no precision; W is quantized to e3m4 with a
    x512 scale so values sit in the e3m4 normal range; max rel err 2^-5,
    measured end-to-end rel err 1.45e-2 incl. bf16 A').
    Streamed as 4 x 1MB delay-pair DMAs alternating the two HWDGE queues.
  - A' (128 x 2048 bf16 = 0.5MB) is built host-side and DMA'd once.
  - PE orientation (measured): self-loading matmuls pay LDWEIGHTS ~= K=128
    cycles UNHIDDEN per matmul, so the big fp8 tensor goes in as the
    STATIONARY operand ([K=128 e, M=128 o-half], 2 PSUM chains) and the
    tiny A' slice streams as the moving operand (N=16): 256 matmuls x
    (128 ldw + 16 stream + ~17 ovh) ~= 17us, vs 23us for the reverse
    orientation (128 matmuls x (128 + 256 + ovh)).
  - The two o-half PSUM chains run as two grouped sweeps (alternating
    banks between consecutive matmuls measured ~2.5us slower).
  - For_i timing loop: 2 bodies per hardware-loop iteration, because
    SBUF addresses inside For_i are fixed -- pools only double-buffer
    across unrolled copies.
"""

import numpy as np

import concourse.bass as bass  # noqa: F401
import concourse.mybir as mybir
from concourse import bacc
from concourse.bass_utils import run_bass_kernel_spmd
from concourse.tile import TileContext

D, B, E, O = 8, 16, 2048, 2048
NCORES = 8
P = 128
O_SH = O // NCORES  # 256 post columns per core
EC = E // P  # 16 e-chunks
SW = 512.0  # fp8 weight scale (folded back via A' /= SW)

_NC_CACHE = {}


def _build(loop_iters=None):
    f32 = mybir.dt.float32
    bf16 = mybir.dt.bfloat16
    fp8 = mybir.dt.float8e3

    nc = bacc.Bacc("TRN2", target_bir_lowering=False, debug=False)
    x_m8 = nc.dram_tensor("m8", [D, P, EC * O_SH], fp8, kind="ExternalInput")
    x_a = nc.dram_tensor("a", [P, EC * D * B], bf16, kind="ExternalInput")
    # y[p, oh, b] = I[b, oh*128 + p]  (o-half on free dim, post on partitions)
    y = nc.dram_tensor("y", [P, 2 * B], f32, kind="ExternalOutput")

    with TileContext(nc) as tc:
        with (
            tc.tile_pool(name="ap_", bufs=2) as ap_,
            tc.tile_pool(name="m8p", bufs=8) as m8p,
            tc.tile_pool(name="psp", bufs=4, space="PSUM") as psp,
            tc.tile_pool(name="outp", bufs=2) as outp,
        ):

            def body(_i=None):
                a_t = ap_.tile([P, EC, D, B], bf16, tag="a")
                nc.scalar.dma_start(out=a_t[:], in_=x_a.ap())
                ps0 = psp.tile([P, B], f32, tag="ps0")
                ps1 = psp.tile([P, B], f32, tag="ps1")
                m_ts = []
                for g in range(4):
                    m_t = m8p.tile([P, 2, EC, O_SH], fp8, tag="m8")
                    eng = nc.scalar if g % 2 else nc.sync
                    src = x_m8.ap()[2 * g : 2 * g + 2]
                    eng.dma_start(out=m_t[:], in_=src.rearrange("d p x -> p d x"))
                    m_ts.append(m_t)
                    # first sweep (o-half 0) tracks the DMA stream
                    for dd in range(2):
                        d = 2 * g + dd
                        for c in range(EC):
                            nc.tensor.matmul(
                                ps0[:],
                                m_t[:, dd, c, 0:P],
                                a_t[:, c, d, :],
                                start=(d == 0 and c == 0),
                                stop=(d == D - 1 and c == EC - 1),
                            )
                # second sweep (o-half 1) over the resident tiles
                for g in range(4):
                    for dd in range(2):
                        d = 2 * g + dd
                        for c in range(EC):
                            nc.tensor.matmul(
                                ps1[:],
                                m_ts[g][:, dd, c, P : 2 * P],
                                a_t[:, c, d, :],
                                start=(d == 0 and c == 0),
                                stop=(d == D - 1 and c == EC - 1),
                            )
                o_t = outp.tile([P, 2, B], f32, tag="o")
                nc.vector.tensor_copy(out=o_t[:, 0, :], in_=ps0[:])
                nc.vector.tensor_copy(out=o_t[:, 1, :], in_=ps1[:])
                nc.scalar.dma_start(out=y.ap(), in_=o_t[:])

            if loop_iters is None:
                body()
            else:
                # see docstring: 2 bodies per For_i iteration
                assert loop_iters % 2 == 0
                with tc.For_i(
                    0, loop_iters // 2, 1, hint_engines=(mybir.EngineType.PE,)
                ) as i:
                    body(i)
                    body(i)

    nc.compile()
    return nc


def _get_nc(loop_iters=None):
    key = loop_iters
    if key not in _NC_CACHE:
        _NC_CACHE[key] = _build(loop_iters)
    return _NC_CACHE[key]


def _make_in_maps(W, Xd, delaymap, Wshort, signs_pre):
    import ml_dtypes

    bf16 = ml_dtypes.bfloat16
    e3m4 = ml_dtypes.float8_e3m4
    W = np.asarray(W, dtype=np.float32)
    Xd = np.asarray(Xd, dtype=np.float32)
    delaymap = np.asarray(delaymap)
    Wshort = np.asarray(Wshort, dtype=np.float32)
    signs_pre = np.asarray(signs_pre)

    s = (2 * signs_pre - 1).astype(np.float32)  # (E,)
    # A'[d,b,e] = Xd*(Wshort+1)*s/SW; layout a[p, c, d, b]
    a = (Xd * (Wshort + 1.0)) * (s / SW)[None, None, :]  # (D,B,E)
    a_re = np.ascontiguousarray(
        a.reshape(D, B, EC, P).transpose(3, 2, 0, 1).reshape(P, EC * D * B)
    ).astype(bf16)

    # W quantized once to e3m4 (scaled), then masked per-delay (exact).
    w8 = (W * SW).astype(e3m4).astype(np.float32)  # (E,O)
    dm_b = delaymap != 0  # (D,E,O) binary

    in_maps = []
    for i in range(NCORES):
        o0 = i * O_SH
        m8 = np.where(dm_b[:, :, o0 : o0 + O_SH], w8[None, :, o0 : o0 + O_SH], 0.0)
        # layout m8[d, p, (c, o)]
        m8_re = np.ascontiguousarray(
            m8.reshape(D, EC, P, O_SH)
            .transpose(0, 2, 1, 3)
            .reshape(D, P, EC * O_SH)
            .astype(e3m4)
        )
        in_maps.append({"m8": m8_re, "a": a_re})
    return in_maps


def run(W, Xd, delaymap, Wshort, signs_pre, loop_iters=None):
    """Run on the 8 NeuronCores; returns (I, BassKernelResults)."""
    nc = _get_nc(loop_iters)
    in_maps = _make_in_maps(W, Xd, delaymap, Wshort, signs_pre)
    res = run_bass_kernel_spmd(nc, in_maps, core_ids=list(range(NCORES)))
    parts = []
    for i in range(NCORES):
        yv = res.results[i]["y"].reshape(P, 2, B)  # [p, oh, b]
        parts.append(yv.transpose(2, 1, 0).reshape(B, 2 * P))  # [b, o]
    I = np.concatenate(parts, axis=1).astype(np.float32)
    return I, res


def kernel(W, Xd, delaymap, Wshort, signs_pre):
    I, _ = run(W, Xd, delaymap, Wshort, signs_pre)
    return I
